# revision 8
# baseline (speedup 1.0000x reference)
# Relational GCN message-passing layer (MolGAN-style) on 8 Trainium2 NeuronCores.
# See kernel.py for the algorithm derivation; this revision additionally:
#   * preamble loads (x, weight, theta) go via HWDGE (nc.sync) in fp32 with DVE
#     cast-copies, so the SWDGE/Q7 descriptor generator starts emitting slab
#     descriptors immediately,
#   * post-norm stage 2: the per-relation outputs land UNNORMALIZED in two
#     rotating PSUM banks (diag(n)(m W) == (diag(n) m) W; one accumulation
#     group per bank — a start=True matmul clears has_written for the WHOLE
#     bank, and PE-write + DVE-read of one bank is a fatal HW collision), and
#     the degree normalization is applied per-partition in the combine step
#     (scalar_tensor_tensor: acc = psum_c*nrm_c + acc), so the DVE reciprocal
#     is off the PE critical path,
#   * the theta_root term x @ theta is loop-invariant and precomputed once in
#     the preamble (stage 2 drops to 4 matmuls).

import os
from contextlib import ExitStack

import numpy as np

import concourse.tile as tile
from concourse import bacc, mybir
from concourse.bass_utils import run_bass_kernel_spmd
from concourse.masks import make_identity

S, N, C5, R, CIN, COUT = 16, 1024, 5, 4, 128, 128
NCORES = 8
SPC = S // NCORES  # batches per core
NB = N // 128      # 128-row node blocks
XW = CIN + 2       # x~ row stride: 128 data + 1 ones + 1 pad

F16 = mybir.dt.float16
F32 = mybir.dt.float32

# tuned on HW (see session notes)
CFG = dict(slab_bufs=3, atp_bufs=2, nchunk=4)


def _kernel_body(tc, bench_iters=1, cfg=None):
    cfg = {**CFG, **(cfg or {})}
    nchunk = cfg["nchunk"]
    nc = tc.nc
    A = nc.dram_tensor("A", (SPC, N, N, C5), F32, kind="ExternalInput").ap()
    x = nc.dram_tensor("x", (SPC, N, CIN), F32, kind="ExternalInput").ap()
    w = nc.dram_tensor("weight", (CIN, COUT, R), F32, kind="ExternalInput").ap()
    th = nc.dram_tensor("theta_root", (CIN, COUT), F32, kind="ExternalInput").ap()
    y = nc.dram_tensor("y", (SPC, N, COUT), F32, kind="ExternalOutput").ap()

    with ExitStack() as ctx:
        consts = ctx.enter_context(tc.tile_pool(name="consts", bufs=1))
        slabs = ctx.enter_context(tc.tile_pool(name="slabs", bufs=cfg["slab_bufs"]))
        atp = ctx.enter_context(tc.tile_pool(name="atp", bufs=cfg["atp_bufs"]))
        small = ctx.enter_context(tc.tile_pool(name="small", bufs=3))
        outp = ctx.enter_context(tc.tile_pool(name="outp", bufs=2))
        ptp = ctx.enter_context(tc.tile_pool(name="ptp", bufs=3, space="PSUM"))
        pm = ctx.enter_context(tc.tile_pool(name="pm", bufs=2, space="PSUM"))
        pmt = ctx.enter_context(tc.tile_pool(name="pmt", bufs=1, space="PSUM"))
        po = ctx.enter_context(tc.tile_pool(name="po", bufs=2, space="PSUM"))

        ident = consts.tile([128, 128], F16)
        # memset on DVE so the Q7/SWDGE queue only runs the affine_select
        # before it starts emitting slab descriptors (single-shot ramp)
        nc.vector.memset(ident, 0.0)
        make_identity(nc, ident, nomemset=True)

        # fp32 staging via HWDGE; DVE casts to fp16 working tiles
        wtmp = consts.tile([128, COUT * R], F32)
        nc.sync.dma_start(out=wtmp, in_=w.rearrange("a b c -> a (b c)"))
        w2 = consts.tile([128, R, COUT], F16)
        wv = wtmp.rearrange("a (b c) -> a b c", c=R)
        for c in range(R):
            nc.vector.tensor_copy(out=w2[:, c, :], in_=wv[:, :, c])
        ths = consts.tile([128, COUT], F32)
        nc.sync.dma_start(out=ths, in_=th)
        th16 = consts.tile([128, COUT], F16)
        nc.vector.tensor_copy(out=th16, in_=ths)

        xs = consts.tile([128, SPC * NB, CIN], F32)
        for s in range(SPC):
            nc.sync.dma_start(
                out=xs[:, s * NB : (s + 1) * NB, :],
                in_=x[s].rearrange("(jb p) a -> p jb a", p=128),
            )
        xe = consts.tile([128, SPC * NB, XW], F16)
        nc.vector.memset(xe[:, :, CIN], 1.0)
        nc.vector.tensor_copy(out=xe[:, :, :CIN], in_=xs)
        # xT tiles [a, i] for the theta_root term
        xT = consts.tile([128, SPC * NB, CIN], F16)
        for k in range(SPC * NB):
            pt = pmt.tile([128, 128], F16, tag="mt")
            nc.tensor.transpose(pt, xe[:, k, :CIN], ident)
            nc.vector.tensor_copy(out=xT[:, k, :], in_=pt)
        # loop-invariant theta_root term: xth[i, b] = (x @ theta)[i, b]
        xth = consts.tile([128, SPC * NB, COUT], F32)
        for k in range(SPC * NB):
            pth = pm.tile([128, CIN + 1], F32, tag="m")
            nc.tensor.matmul(
                pth[:, :COUT], lhsT=xT[:, k, :], rhs=th16, start=True, stop=True
            )
            nc.vector.tensor_copy(out=xth[:, k, :], in_=pth[:, :COUT])

        tg_dve = cfg.get("tg_copy", "mixed") == "dve"

        def transpose_group(slab_t, at_t, p):
            ps = ptp.tile([128, 1024], F16, tag="tp")
            for q in range(2):
                jb = 2 * p + q
                for c in range(R):
                    col = q * 512 + c * 128
                    nc.tensor.transpose(
                        ps[:, col : col + 128],
                        slab_t[:, jb * 128 : (jb + 1) * 128, c],
                        ident,
                    )
            dst = at_t[:, p * 1024 : (p + 1) * 1024]
            if tg_dve or p % 2 == 0:
                nc.vector.tensor_copy(out=dst, in_=ps)
            else:
                nc.scalar.copy(out=dst, in_=ps)

        def stage1(si, at_t, c):
            m = pm.tile([128, CIN + 1], F32, tag="m")
            for jb in range(NB):
                nc.tensor.matmul(
                    m,
                    lhsT=at_t[:, jb * 512 + c * 128 : jb * 512 + (c + 1) * 128],
                    rhs=xe[:, si * NB + jb, : CIN + 1],
                    start=(jb == 0),
                    stop=(jb == NB - 1),
                )
            # one ACT copy moves m AND its rowsum to SBUF; the reciprocal
            # then reads SBUF, so ACT/DVE never co-access the m PSUM bank
            mn = small.tile([128, CIN + 1], F16, tag=f"mn{c}")
            nc.scalar.copy(mn, m)
            nrm = small.tile([128, 1], F32, tag=f"norm{c}")
            nc.vector.reciprocal(nrm, mn[:, CIN : CIN + 1])
            pt = pmt.tile([128, 128], F16, tag="mt")
            nc.tensor.transpose(pt, mn[:, :CIN], ident)
            mt = small.tile([128, CIN], F16, tag=f"mts{c}")
            nc.vector.tensor_copy(out=mt, in_=pt)
            return mt, nrm

        def stage2(si, ib, mts):
            # per-relation MMs each own a whole PSUM bank (2 rotating); the
            # combine acc_c = out_c*nrm_c + acc_{c-1} (acc_{-1} = x@theta)
            # reads bank c while the MM for c+1 writes the other bank.
            k = si * NB + ib
            accs = []
            for c in range(R):
                out_ps = po.tile([128, COUT], F32, tag="o")
                nc.tensor.matmul(
                    out_ps, lhsT=mts[c][0], rhs=w2[:, c, :], start=True, stop=True
                )
                acc = small.tile([128, COUT], F32, tag=f"acc{c}")
                prev = xth[:, k, :] if c == 0 else accs[-1]
                nc.vector.scalar_tensor_tensor(
                    out=acc, in0=out_ps, scalar=mts[c][1], in1=prev,
                    op0=mybir.AluOpType.mult, op1=mybir.AluOpType.add,
                )
                accs.append(acc)
            ot = outp.tile([128, COUT], F32, tag="out")
            nc.scalar.activation(ot, accs[-1], mybir.ActivationFunctionType.Tanh)
            nc.sync.dma_start(out=y[si, ib * 128 : (ib + 1) * 128, :], in_=ot)

        def main_pipeline():
            lag = cfg.get("lag", 1)
            hist = []
            csz = N // nchunk
            NT = SPC * NB
            for t in range(NT + lag):
                if t < NT:
                    si, ib = divmod(t, NB)
                    slab_t = slabs.tile([128, N, C5], F16, tag="slab")
                    for p4 in range(nchunk):
                        nc.gpsimd.dma_start(
                            out=slab_t[:, p4 * csz : (p4 + 1) * csz, :],
                            in_=A[
                                si,
                                ib * 128 : (ib + 1) * 128,
                                p4 * csz : (p4 + 1) * csz,
                                :,
                            ],
                        )
                    at_t = atp.tile([128, NB * R * 128], F16, tag="at")
                    hist.append((si, ib, at_t))
                prev = hist[t - lag] if t >= lag else None
                mts = []
                for p in range(4):
                    if t < NT:
                        transpose_group(slab_t, at_t, p)
                    if prev is not None:
                        mts.append(stage1(prev[0], prev[2], p))
                if prev is not None:
                    stage2(prev[0], prev[1], mts)

        ndup = cfg.get("dup", 1)
        if bench_iters > 1:
            with tc.For_i(
                0,
                bench_iters,
                1,
                hint_engines=(
                    mybir.EngineType.PE,
                    mybir.EngineType.DVE,
                    mybir.EngineType.Activation,
                    mybir.EngineType.Pool,
                ),
            ):
                for _ in range(ndup):
                    main_pipeline()
        else:
            main_pipeline()


_CACHE = {}


def build_nc(bench_iters=1, cfg=None):
    nc = bacc.Bacc(
        "TRN2", target_bir_lowering=False, debug=False, num_devices=NCORES
    )
    with tile.TileContext(nc) as tc:
        _kernel_body(tc, bench_iters, cfg)
    nc.compile()
    return nc


def _get_nc():
    if "nc" not in _CACHE:
        _CACHE["nc"] = build_nc(1)
    return _CACHE["nc"]


LAST = None


class Variant:
    """profile_hw-compatible wrapper for a config sweep."""

    NCORES = NCORES
    SPC = SPC

    def __init__(self, **cfg):
        self.cfg = cfg
        self.variant = str(sorted(cfg.items()))

    def build_nc(self, bench_iters=1):
        return build_nc(bench_iters, self.cfg)


def kernel(A, x, weight, theta_root):
    global LAST
    A = np.ascontiguousarray(np.asarray(A), dtype=np.float32)
    x = np.ascontiguousarray(np.asarray(x), dtype=np.float32)
    weight = np.ascontiguousarray(np.asarray(weight), dtype=np.float32)
    theta_root = np.ascontiguousarray(np.asarray(theta_root), dtype=np.float32)
    os.environ["BASS_NEVER_TRACE"] = "1"
    nc = _get_nc()
    in_maps = []
    for k in range(NCORES):
        sl = slice(k * SPC, (k + 1) * SPC)
        in_maps.append(
            {
                "A": np.ascontiguousarray(A[sl]),
                "x": np.ascontiguousarray(x[sl]),
                "weight": weight,
                "theta_root": theta_root,
            }
        )
    res = run_bass_kernel_spmd(nc, in_maps, core_ids=list(range(NCORES)))
    LAST = res
    return np.concatenate([r["y"] for r in res.results], axis=0)


# revision 12
# speedup vs baseline: 1.0109x; 1.0109x over previous
# Relational GCN message-passing layer (MolGAN-style) on 8 Trainium2 NeuronCores.
# See kernel.py for the algorithm derivation; this revision additionally:
#   * preamble loads (x, weight, theta) go via HWDGE (nc.sync) in fp32 with DVE
#     cast-copies, so the SWDGE/Q7 descriptor generator starts emitting slab
#     descriptors immediately,
#   * post-norm stage 2: the per-relation outputs land UNNORMALIZED in two
#     rotating PSUM banks (diag(n)(m W) == (diag(n) m) W; one accumulation
#     group per bank — a start=True matmul clears has_written for the WHOLE
#     bank, and PE-write + DVE-read of one bank is a fatal HW collision), and
#     the degree normalization is applied per-partition in the combine step
#     (scalar_tensor_tensor: acc = psum_c*nrm_c + acc), so the DVE reciprocal
#     is off the PE critical path,
#   * the theta_root term x @ theta is loop-invariant and precomputed once in
#     the preamble (stage 2 drops to 4 matmuls).

import os
from contextlib import ExitStack

import numpy as np

import concourse.tile as tile
from concourse import bacc, mybir
from concourse.bass_utils import run_bass_kernel_spmd
from concourse.masks import make_identity

S, N, C5, R, CIN, COUT = 16, 1024, 5, 4, 128, 128
NCORES = 8
SPC = S // NCORES  # batches per core
NB = N // 128      # 128-row node blocks
XW = CIN + 2       # x~ row stride: 128 data + 1 ones + 1 pad

F16 = mybir.dt.float16
F32 = mybir.dt.float32

# tuned on HW (see session notes)
CFG = dict(slab_bufs=3, atp_bufs=2, nchunk=4)


def _kernel_body(tc, bench_iters=1, cfg=None):
    cfg = {**CFG, **(cfg or {})}
    nchunk = cfg["nchunk"]
    nc = tc.nc
    A = nc.dram_tensor("A", (SPC, N, N, C5), F32, kind="ExternalInput").ap()
    x = nc.dram_tensor("x", (SPC, N, CIN), F32, kind="ExternalInput").ap()
    w = nc.dram_tensor("weight", (CIN, COUT, R), F32, kind="ExternalInput").ap()
    th = nc.dram_tensor("theta_root", (CIN, COUT), F32, kind="ExternalInput").ap()
    y = nc.dram_tensor("y", (SPC, N, COUT), F32, kind="ExternalOutput").ap()

    with ExitStack() as ctx:
        consts = ctx.enter_context(tc.tile_pool(name="consts", bufs=1))
        slabs = ctx.enter_context(tc.tile_pool(name="slabs", bufs=cfg["slab_bufs"]))
        atp = ctx.enter_context(tc.tile_pool(name="atp", bufs=cfg["atp_bufs"]))
        small = ctx.enter_context(tc.tile_pool(name="small", bufs=3))
        outp = ctx.enter_context(tc.tile_pool(name="outp", bufs=2))
        ptp = ctx.enter_context(tc.tile_pool(name="ptp", bufs=3, space="PSUM"))
        pm = ctx.enter_context(tc.tile_pool(name="pm", bufs=2, space="PSUM"))
        pmt = ctx.enter_context(tc.tile_pool(name="pmt", bufs=1, space="PSUM"))
        po = ctx.enter_context(tc.tile_pool(name="po", bufs=2, space="PSUM"))

        def load_slab(si, ib):
            csz = N // nchunk
            slab_t = slabs.tile([128, N, C5], F16, tag="slab")
            for p4 in range(nchunk):
                nc.gpsimd.dma_start(
                    out=slab_t[:, p4 * csz : (p4 + 1) * csz, :],
                    in_=A[
                        si,
                        ib * 128 : (ib + 1) * 128,
                        p4 * csz : (p4 + 1) * csz,
                        :,
                    ],
                )
            return slab_t

        # Single-shot: emit the first two slabs' loads before ANY other
        # Q7/const work, so the A stream starts at t~0 (the Pool queue
        # otherwise leads with the identity affine_select).  The bench-loop
        # path takes no peel: every iteration loads all 16 slabs itself.
        peeled = []
        if bench_iters == 1:
            peeled = [load_slab(*divmod(t0, NB)) for t0 in range(2)]

        ident = consts.tile([128, 128], F16)
        # memset on DVE so the Q7/SWDGE queue only runs the affine_select
        # before it starts emitting slab descriptors (single-shot ramp)
        nc.vector.memset(ident, 0.0)
        make_identity(nc, ident, nomemset=True)

        # fp32 staging via HWDGE; DVE casts to fp16 working tiles
        wtmp = consts.tile([128, COUT * R], F32)
        nc.sync.dma_start(out=wtmp, in_=w.rearrange("a b c -> a (b c)"))
        w2 = consts.tile([128, R, COUT], F16)
        wv = wtmp.rearrange("a (b c) -> a b c", c=R)
        for c in range(R):
            nc.vector.tensor_copy(out=w2[:, c, :], in_=wv[:, :, c])
        ths = consts.tile([128, COUT], F32)
        nc.sync.dma_start(out=ths, in_=th)
        th16 = consts.tile([128, COUT], F16)
        nc.vector.tensor_copy(out=th16, in_=ths)

        xs = consts.tile([128, SPC * NB, CIN], F32)
        for s in range(SPC):
            nc.sync.dma_start(
                out=xs[:, s * NB : (s + 1) * NB, :],
                in_=x[s].rearrange("(jb p) a -> p jb a", p=128),
            )
        xe = consts.tile([128, SPC * NB, XW], F16)
        nc.vector.memset(xe[:, :, CIN], 1.0)
        nc.vector.tensor_copy(out=xe[:, :, :CIN], in_=xs)
        # xT tiles [a, i] for the theta_root term
        xT = consts.tile([128, SPC * NB, CIN], F16)
        for k in range(SPC * NB):
            pt = pmt.tile([128, 128], F16, tag="mt")
            nc.tensor.transpose(pt, xe[:, k, :CIN], ident)
            nc.vector.tensor_copy(out=xT[:, k, :], in_=pt)
        # loop-invariant theta_root term: xth[i, b] = (x @ theta)[i, b]
        xth = consts.tile([128, SPC * NB, COUT], F32)
        for k in range(SPC * NB):
            pth = pm.tile([128, CIN + 1], F32, tag="m")
            nc.tensor.matmul(
                pth[:, :COUT], lhsT=xT[:, k, :], rhs=th16, start=True, stop=True
            )
            nc.vector.tensor_copy(out=xth[:, k, :], in_=pth[:, :COUT])

        tg_dve = cfg.get("tg_copy", "mixed") == "dve"

        def transpose_group(slab_t, at_t, p):
            ps = ptp.tile([128, 1024], F16, tag="tp")
            for q in range(2):
                jb = 2 * p + q
                for c in range(R):
                    col = q * 512 + c * 128
                    nc.tensor.transpose(
                        ps[:, col : col + 128],
                        slab_t[:, jb * 128 : (jb + 1) * 128, c],
                        ident,
                    )
            dst = at_t[:, p * 1024 : (p + 1) * 1024]
            if tg_dve or p % 2 == 0:
                nc.vector.tensor_copy(out=dst, in_=ps)
            else:
                nc.scalar.copy(out=dst, in_=ps)

        def stage1(si, at_t, c):
            m = pm.tile([128, CIN + 1], F32, tag="m")
            for jb in range(NB):
                nc.tensor.matmul(
                    m,
                    lhsT=at_t[:, jb * 512 + c * 128 : jb * 512 + (c + 1) * 128],
                    rhs=xe[:, si * NB + jb, : CIN + 1],
                    start=(jb == 0),
                    stop=(jb == NB - 1),
                )
            # one ACT copy moves m AND its rowsum to SBUF; the reciprocal
            # then reads SBUF, so ACT/DVE never co-access the m PSUM bank
            mn = small.tile([128, CIN + 1], F16, tag=f"mn{c}")
            nc.scalar.copy(mn, m)
            nrm = small.tile([128, 1], F32, tag=f"norm{c}")
            nc.vector.reciprocal(nrm, mn[:, CIN : CIN + 1])
            pt = pmt.tile([128, 128], F16, tag="mt")
            nc.tensor.transpose(pt, mn[:, :CIN], ident)
            mt = small.tile([128, CIN], F16, tag=f"mts{c}")
            nc.vector.tensor_copy(out=mt, in_=pt)
            return mt, nrm

        def stage2(si, ib, mts):
            # per-relation MMs each own a whole PSUM bank (2 rotating); the
            # combine acc_c = out_c*nrm_c + acc_{c-1} (acc_{-1} = x@theta)
            # reads bank c while the MM for c+1 writes the other bank.
            k = si * NB + ib
            accs = []
            for c in range(R):
                out_ps = po.tile([128, COUT], F32, tag="o")
                nc.tensor.matmul(
                    out_ps, lhsT=mts[c][0], rhs=w2[:, c, :], start=True, stop=True
                )
                acc = small.tile([128, COUT], F32, tag=f"acc{c}")
                prev = xth[:, k, :] if c == 0 else accs[-1]
                nc.vector.scalar_tensor_tensor(
                    out=acc, in0=out_ps, scalar=mts[c][1], in1=prev,
                    op0=mybir.AluOpType.mult, op1=mybir.AluOpType.add,
                )
                accs.append(acc)
            ot = outp.tile([128, COUT], F32, tag="out")
            nc.scalar.activation(ot, accs[-1], mybir.ActivationFunctionType.Tanh)
            nc.sync.dma_start(out=y[si, ib * 128 : (ib + 1) * 128, :], in_=ot)

        def main_pipeline(preloaded=()):
            lag = cfg.get("lag", 1)
            hist = []
            NT = SPC * NB
            for t in range(NT + lag):
                if t < NT:
                    si, ib = divmod(t, NB)
                    if t < len(preloaded):
                        slab_t = preloaded[t]
                    else:
                        slab_t = load_slab(si, ib)
                    at_t = atp.tile([128, NB * R * 128], F16, tag="at")
                    hist.append((si, ib, at_t))
                prev = hist[t - lag] if t >= lag else None
                mts = []
                for p in range(4):
                    if t < NT:
                        transpose_group(slab_t, at_t, p)
                    if prev is not None:
                        mts.append(stage1(prev[0], prev[2], p))
                if prev is not None:
                    stage2(prev[0], prev[1], mts)

        ndup = cfg.get("dup", 1)
        if bench_iters > 1:
            with tc.For_i(
                0,
                bench_iters,
                1,
                hint_engines=(
                    mybir.EngineType.PE,
                    mybir.EngineType.DVE,
                    mybir.EngineType.Activation,
                    mybir.EngineType.Pool,
                ),
            ):
                for _ in range(ndup):
                    main_pipeline()
        else:
            main_pipeline(peeled)


_CACHE = {}


def build_nc(bench_iters=1, cfg=None):
    nc = bacc.Bacc(
        "TRN2", target_bir_lowering=False, debug=False, num_devices=NCORES
    )
    with tile.TileContext(nc) as tc:
        _kernel_body(tc, bench_iters, cfg)
    nc.compile()
    return nc


def _get_nc():
    if "nc" not in _CACHE:
        _CACHE["nc"] = build_nc(1)
    return _CACHE["nc"]


LAST = None


class Variant:
    """profile_hw-compatible wrapper for a config sweep."""

    NCORES = NCORES
    SPC = SPC

    def __init__(self, **cfg):
        self.cfg = cfg
        self.variant = str(sorted(cfg.items()))

    def build_nc(self, bench_iters=1):
        return build_nc(bench_iters, self.cfg)


def kernel(A, x, weight, theta_root):
    global LAST
    A = np.ascontiguousarray(np.asarray(A), dtype=np.float32)
    x = np.ascontiguousarray(np.asarray(x), dtype=np.float32)
    weight = np.ascontiguousarray(np.asarray(weight), dtype=np.float32)
    theta_root = np.ascontiguousarray(np.asarray(theta_root), dtype=np.float32)
    os.environ["BASS_NEVER_TRACE"] = "1"
    nc = _get_nc()
    in_maps = []
    for k in range(NCORES):
        sl = slice(k * SPC, (k + 1) * SPC)
        in_maps.append(
            {
                "A": np.ascontiguousarray(A[sl]),
                "x": np.ascontiguousarray(x[sl]),
                "weight": weight,
                "theta_root": theta_root,
            }
        )
    res = run_bass_kernel_spmd(nc, in_maps, core_ids=list(range(NCORES)))
    LAST = res
    return np.concatenate([r["y"] for r in res.results], axis=0)
